# revision 1
# baseline (speedup 1.0000x reference)
"""Tensor-parallel causal attention block (qkv proj + RoPE + attention + out proj)
for Trainium2, sharded over 8 NeuronCores by attention head (2 heads/core).

Contract: kernel(**inputs) takes the FULL inputs (x [1,2048,1024] f32,
w_in [3072,1024] f32, w_out [1024,1024] f32, is_causal scalar) and returns the
FULL output [1,2048,1024] f32.

Per-core layout strategy (everything kept transposed, [feature, seq], so no
on-device transposes of activations are ever needed):
  - host pre-transposes x -> xT [1024,2048] and the weight shards
  - qkvT = w_shard @ xT  ->  [384, 2048] (Q.T | K.T | V.T rows, 2 heads packed)
  - RoPE applied in [hd, s] layout via a constant rotation matmul + elementwise
  - scores computed transposed: S.T[k, q] = K @ Q.T  (softmax dim = partitions)
  - probs (unnormalized exp) hit PV directly:  ctx.T = V_aug.T @ P.T, where
    V_aug carries a ones column so row 64 of the PV output is the softmax
    denominator; normalization happens after PV via reciprocal + PE broadcast
  - out partial = ctx @ w_out_shard.T; host sums the 8 partials (the TP
    all-reduce is a plain numpy sum of disjoint-head partials).
All matmuls run in float32r (full-rate fp32 on the PE, ~1e-4 relative).
"""
import sys

sys.path.insert(0, '/opt/trn_rl_repo')

from contextlib import ExitStack

import numpy as np

import concourse.bass as bass
from concourse import mybir, tile
from concourse.bass_utils import run_bass_kernel_spmd

B, S, D, H = 1, 2048, 1024, 16
HD = D // H            # 64
NCORES = 8
HPC = H // NCORES      # heads per core = 2
EPC = HPC * HD         # features per core = 128
ROPE_BASE = 10000.0

F32 = mybir.dt.float32
F32R = mybir.dt.float32r

QC = 512               # q-chunk width (one PSUM bank of fp32)
NQC = S // QC          # 4 q-chunks
NST = S // 128         # 16 s-tiles / k-tiles
ND = D // 128          # 8 contraction tiles for the input projection


def _split_multi_waits(nc, max_waits=1):
    """This container's walrus build accepts at most one embedded sync wait per
    instruction; move extra waits onto preceding same-engine NoOps."""
    n_split = 0
    for fn in nc.m.functions:
        for blk in fn.blocks:
            new_insts = []
            for inst in blk.instructions:
                si = inst.sync_info
                waits = list(si.on_wait) if (si and si.on_wait) else []
                if len(waits) > max_waits and inst.engine is not None:
                    for w in waits[max_waits:]:
                        nop = mybir.InstNoOp(
                            name=f"{inst.name}_wn{n_split}", ins=[], outs=[])
                        n_split += 1
                        nop.engine = inst.engine
                        nop.sync_info = mybir.SyncInfo(on_wait=[w], on_update=[])
                        nc.register_instruction(nop, overwrite=True)
                        new_insts.append(nop)
                    si.on_wait = waits[:max_waits]
                new_insts.append(inst)
            blk.instructions[:] = new_insts
    return n_split


def _host_constants():
    inv_freq = 1.0 / (ROPE_BASE ** (np.arange(0, HD, 2, dtype=np.float64) / HD))
    t = np.arange(S, dtype=np.float64)
    freqs = np.outer(inv_freq, t)                    # [32, S]  ([hd, s] layout)
    emb = np.concatenate([freqs, freqs], axis=0)     # [64, S]
    cosT = np.cos(emb).astype(np.float32)
    sinT = np.sin(emb).astype(np.float32)
    cos2 = np.tile(cosT, (2, 1))                     # [128, S] (2 heads packed)
    sin2 = np.tile(sinT, (2, 1))
    # rotate_half as a matrix: (R q)[i] = -q[i+32] (i<32), q[i-32] (i>=32)
    R = np.zeros((HD, HD), dtype=np.float32)
    for i in range(HD // 2):
        R[i, i + HD // 2] = -1.0
        R[i + HD // 2, i] = 1.0
    R2 = np.zeros((128, 128), dtype=np.float32)
    R2[0:64, 0:64] = R
    R2[64:128, 64:128] = R
    rotT = np.ascontiguousarray(R2.T)
    # upper-triangular (k<=q) mask for the diagonal 128x128 blocks of S.T[k,q]
    tri = np.triu(np.ones((128, 128), dtype=np.float32))
    ident = np.eye(128, dtype=np.float32)
    return cos2, sin2, rotT, tri, ident


def _build_program(causal: bool):
    nc = bass.Bass()
    xT_d = nc.dram_tensor("xT", [D, S], F32R, kind="ExternalInput")
    winT_d = nc.dram_tensor("winT", [D, 3 * EPC], F32R, kind="ExternalInput")
    woT0_d = nc.dram_tensor("woT0", [HD, D], F32R, kind="ExternalInput")
    woT1_d = nc.dram_tensor("woT1", [HD, D], F32R, kind="ExternalInput")
    pout_d = nc.dram_tensor("pout", [S, D], F32, kind="ExternalOutput")

    cos2_np, sin2_np, rotT_np, tri_np, ident_np = _host_constants()
    cos2_d = nc.inline_tensor(cos2_np, name="cos2")
    sin2_d = nc.inline_tensor(sin2_np, name="sin2")
    rotT_d = nc.dram_tensor("rotT", [128, 128], F32R, kind="ExternalInput")
    tri_d = nc.dram_tensor("tri", [128, 128], F32R, kind="ExternalInput")
    ident_d = nc.dram_tensor("ident", [128, 128], F32R, kind="ExternalInput")

    with tile.TileContext(nc) as tc, ExitStack() as ctx:
        sb = ctx.enter_context(tc.tile_pool(name="sb", bufs=1))
        wk0 = ctx.enter_context(tc.tile_pool(name="wk0", bufs=1))

        # ---- persistent SBUF tensors -----------------------------------
        winT = sb.tile([128, ND * 3 * EPC], F32R, name="winT")
        for d in range(ND):
            nc.sync.dma_start(winT[:, d * 3 * EPC:(d + 1) * 3 * EPC],
                              winT_d[d * 128:(d + 1) * 128, :])
        woT0 = sb.tile([HD, D], F32R, name="woT0")
        nc.sync.dma_start(woT0[:], woT0_d[:, :])
        woT1 = sb.tile([HD, D], F32R, name="woT1")
        nc.sync.dma_start(woT1[:], woT1_d[:, :])
        cos2 = sb.tile([128, S], F32, name="cos2")
        nc.sync.dma_start(cos2[:], cos2_d[:, :])
        sin2 = sb.tile([128, S], F32, name="sin2")
        nc.sync.dma_start(sin2[:], sin2_d[:, :])
        rot = sb.tile([128, 128], F32R, name="rot")
        nc.sync.dma_start(rot[:], rotT_d[:, :])
        tri = sb.tile([128, 128], F32R, name="tri")
        nc.sync.dma_start(tri[:], tri_d[:, :])
        ident = sb.tile([128, 128], F32R, name="ident")
        nc.sync.dma_start(ident[:], ident_d[:, :])
        onesf = sb.tile([128, HD], F32, name="onesf")
        nc.vector.memset(onesf[:], 1.0)

        qraw = sb.tile([128, S], F32R, name="qraw")
        kraw = sb.tile([128, S], F32R, name="kraw")
        vtr = sb.tile([128, S], F32R, name="vtr")
        qrot = sb.tile([128, S], F32R, name="qrot")
        krot = sb.tile([128, S], F32R, name="krot")
        vnat = sb.tile([128, NST * 130], F32R, name="vnat")
        nc.vector.memset(vnat[:].bitcast(F32), 1.0)
        heatout = sb.tile([1, 1], F32, name="heatout")
        ctx0 = sb.tile([HD, S], F32R, name="ctx0")
        ctx1 = sb.tile([HD, S], F32R, name="ctx1")

        def rope_chunk(c0, pfn):
            for (raw, out) in ((qraw, qrot), (kraw, krot)):
                rp = pfn()
                nc.tensor.matmul(rp[:, 0:QC], rot[:], raw[:, c0:c0 + QC],
                                 start=True, stop=True)
                t1 = wk0.tile([128, QC], F32R, tag="t1", bufs=3, name="t1")
                nc.vector.tensor_mul(t1[:], raw[:, c0:c0 + QC],
                                     cos2[:, c0:c0 + QC])
                t2 = wk0.tile([128, QC], F32R, tag="t2", bufs=3, name="t2")
                nc.vector.tensor_mul(t2[:], rp[:, 0:QC], sin2[:, c0:c0 + QC])
                nc.vector.tensor_add(out[:, c0:c0 + QC], t1[:], t2[:])

        def vt_tile(j, vfn):
            vp = vfn()
            nc.tensor.transpose(vp[:, 0:128], vtr[:, j * 128:(j + 1) * 128],
                                ident[:])
            nc.vector.tensor_copy(vnat[:, j * 130:j * 130 + 64], vp[:, 0:64])
            nc.vector.tensor_copy(vnat[:, j * 130 + 65:j * 130 + 129],
                                  vp[:, 64:128])

        # ========== Stage A, first half (s in [0, 1024)) ================
        with tc.tile_pool(name="psA", bufs=1, space="PSUM") as psA, \
             tc.tile_pool(name="wka", bufs=3) as wka:
            accs = [psA.tile([128, QC], F32, tag="acc", bufs=6, name="acc")
                    for _ in range(6)]
            for d in range(ND):
                xt = wka.tile([128, S // 2], F32R, tag="xt", name="xt")
                for half in range(2):
                    nc.sync.dma_start(
                        xt[:, half * QC:(half + 1) * QC],
                        xT_d[d * 128:(d + 1) * 128,
                             half * QC:(half + 1) * QC])
                for et in range(3):
                    lw = winT[:, d * 3 * EPC + et * 128:
                              d * 3 * EPC + (et + 1) * 128]
                    for sch in range(2):
                        nc.tensor.matmul(
                            accs[et * 2 + sch][:], lw,
                            xt[:, sch * QC:(sch + 1) * QC],
                            start=(d == 0), stop=(d == ND - 1))
            for sch in range(2):
                c0 = sch * QC
                nc.scalar.copy(qraw[:, c0:c0 + QC], accs[0 * 2 + sch][:])
                nc.scalar.copy(kraw[:, c0:c0 + QC], accs[1 * 2 + sch][:])
                nc.scalar.copy(vtr[:, c0:c0 + QC], accs[2 * 2 + sch][:])

            def pa_tile():
                return psA.tile([128, QC], F32, tag="acc", bufs=6, name="rp")

            def pa_vt():
                return psA.tile([128, 128], F32R, tag="acc", bufs=6,
                                name="vp")
            for sch in range(2):
                rope_chunk(sch * QC, pa_tile)
            for j in range(8):
                vt_tile(j, pa_vt)

        # ===== Stage B/C with stage-A-half-2 as filler thunks ===========
        with tc.tile_pool(name="psB", bufs=1, space="PSUM") as psB, \
             tc.tile_pool(name="wkb", bufs=3) as wkb:

            def op_tile():
                return psB.tile([128, QC], F32, tag="op", bufs=2, name="op")

            def op_vt():
                return psB.tile([128, 128], F32R, tag="op", bufs=2, name="vp")

            # issue all xT second-half loads now; they stream in during the
            # first attention chunks
            xts = {0: [], 1: []}
            for sch in range(2):
                for d in range(ND):
                    xt = wkb.tile([128, QC], F32R, tag="xt2", bufs=16,
                                  name="xt2")
                    c0 = S // 2 + sch * QC
                    nc.sync.dma_start(
                        xt[:], xT_d[d * 128:(d + 1) * 128, c0:c0 + QC])
                    xts[sch].append(xt)

            # ---- stage-A second-half work, chopped into filler thunks ----
            acc_h = {}

            def qkv_filler(sch, et, lohi):
                def f():
                    dlo, dhi = lohi
                    c0 = S // 2 + sch * QC
                    if dlo == 0:
                        acc_h[(sch, et)] = op_tile()
                    acc = acc_h[(sch, et)]
                    for d in range(dlo, dhi):
                        lw = winT[:, d * 3 * EPC + et * 128:
                                  d * 3 * EPC + (et + 1) * 128]
                        nc.tensor.matmul(acc[:], lw, xts[sch][d][:],
                                         start=(d == 0), stop=(d == ND - 1))
                    if dhi == ND:
                        dst = (qraw, kraw, vtr)[et]
                        nc.scalar.copy(dst[:, c0:c0 + QC], acc[:])
                        del acc_h[(sch, et)]
                return f

            fillers = []
            for sch in range(2):
                for et in range(3):
                    fillers.append(qkv_filler(sch, et, (0, 4)))
                    fillers.append(qkv_filler(sch, et, (4, ND)))
                    if et == 1:
                        c0 = S // 2 + sch * QC
                        fillers.append(
                            lambda c0=c0: rope_chunk(c0, op_tile))
                for j in range(8 + sch * 4, 12 + sch * 4):
                    fillers.append(lambda j=j: vt_tile(j, op_vt))

            def norm_thunks(qc, pvsb):
                thunks = []
                rcps = []
                for hh in range(2):
                    rcp = wkb.tile([65, QC], F32, tag="rcp", bufs=4,
                                   name="rcp")
                    with nc.allow_low_precision(reason="denom recip"):
                        nc.vector.reciprocal(rcp[64:65, :],
                                             pvsb[hh][64:65, :])
                    rcps.append(rcp)

                def norm(hh):
                    def f():
                        q0 = qc * QC
                        rb = op_tile()
                        nc.tensor.matmul(rb[0:HD, :], onesf[64:65, 0:HD],
                                         rcps[hh][64:65, :],
                                         start=True, stop=True)
                        rbs = wkb.tile([HD, QC], F32, tag="rbs", bufs=2,
                                       name="rbs")
                        nc.scalar.copy(rbs[:], rb[0:HD, :])
                        ctxh = ctx0 if hh == 0 else ctx1
                        nc.vector.tensor_mul(ctxh[:, q0:q0 + QC],
                                             pvsb[hh][0:64, :], rbs[:])
                    return f

                def oproj(sti):
                    def f():
                        c0 = (qc * 4 + sti) * 128
                        ob = wkb.tile([128, D], F32, tag="ob", bufs=3,
                                      name="ob")
                        for dc in range(2):
                            op = op_tile()
                            nc.tensor.matmul(op[:], ctx0[:, c0:c0 + 128],
                                             woT0[:, dc * QC:(dc + 1) * QC],
                                             start=True, stop=False)
                            nc.tensor.matmul(op[:], ctx1[:, c0:c0 + 128],
                                             woT1[:, dc * QC:(dc + 1) * QC],
                                             start=False, stop=True)
                            if dc == 0:
                                nc.scalar.copy(ob[:, dc * QC:(dc + 1) * QC],
                                               op[:])
                            else:
                                nc.vector.tensor_copy(
                                    ob[:, dc * QC:(dc + 1) * QC], op[:])
                        nc.sync.dma_start(pout_d[c0:c0 + 128, :], ob[:])
                    return f

                thunks.append(norm(0))
                thunks.append(norm(1))
                for sti in range(4):
                    thunks.append(oproj(sti))
                return thunks

            LAG = 2

            def attention_chunk(qc, deferred, use_fillers):
                q0 = qc * QC
                n_k = 4 * (qc + 1) if causal else NST
                pvs = [psB.tile([65, QC], F32, tag="pv", bufs=2,
                                name=f"pv{hh}") for hh in range(2)]
                window = []

                def emit_pv(pkt, p0, last):
                    js = max(0, pkt - qc * 4) * 128 if causal else 0
                    for hh in range(2):
                        nc.tensor.matmul(
                            pvs[hh][:, js:QC],
                            vnat[:, pkt * 130 + hh * 65:
                                 pkt * 130 + hh * 65 + 65],
                            p0[:, hh * QC + js:hh * QC + QC],
                            start=(pkt == 0), stop=last)

                for kt in range(n_k):
                    st = psB.tile([128, 2 * QC], F32, tag="st", bufs=2,
                                  name="st")
                    for hh in range(2):
                        nc.tensor.matmul(
                            st[:, hh * QC:(hh + 1) * QC],
                            krot[hh * 64:(hh + 1) * 64,
                                 kt * 128:(kt + 1) * 128],
                            qrot[hh * 64:(hh + 1) * 64, q0:q0 + QC],
                            start=True, stop=True)
                    pt = wkb.tile([128, 2 * QC], F32R, tag="pt", bufs=4,
                                  name="pt")
                    j = kt - qc * 4
                    if causal and j >= 0:
                        for hh in range(2):
                            nc.scalar.activation(
                                pt[:, hh * QC + j * 128:(hh + 1) * QC],
                                st[:, hh * QC + j * 128:(hh + 1) * QC],
                                mybir.ActivationFunctionType.Exp, scale=0.125)
                            nc.vector.tensor_mul(
                                pt[:, hh * QC + j * 128:
                                   hh * QC + (j + 1) * 128],
                                pt[:, hh * QC + j * 128:
                                   hh * QC + (j + 1) * 128], tri[:])
                    else:
                        nc.scalar.activation(
                            pt[:], st[:],
                            mybir.ActivationFunctionType.Exp, scale=0.125)
                    window.append((kt, pt))
                    if len(window) > LAG:
                        emit_pv(*window.pop(0), last=False)
                    if deferred and kt >= 3 and kt % 2 == 1:
                        deferred.pop(0)()
                    if use_fillers and fillers:
                        fillers.pop(0)()
                        if use_fillers > 1 and fillers:
                            fillers.pop(0)()
                while window:
                    kt_, p_ = window.pop(0)
                    emit_pv(kt_, p_, last=(kt_ == n_k - 1))
                while deferred:
                    deferred.pop(0)()
                # evict PV accumulators to SBUF, freeing the PSUM banks
                pvsb = []
                for hh in range(2):
                    pb = wkb.tile([65, QC], F32, tag="pvsb", bufs=4,
                                  name="pvsb")
                    nc.vector.tensor_copy(pb[:], pvs[hh][:])
                    pvsb.append(pb)
                return pvsb

            # warm-up burst bridging the rope->attention dependency gap
            heat = psB.tile([128, 2 * QC], F32, tag="st", bufs=2, name="heat")
            for _ in range(16):
                nc.tensor.matmul(heat[:, 0:QC], winT[:, 0:128],
                                 winT[:, 1024:1536], start=True, stop=True)
            nc.scalar.copy(heatout[:], heat[0:1, 0:1])

            sb0 = attention_chunk(0, [], use_fillers=0)
            sb1 = attention_chunk(1, norm_thunks(0, sb0), use_fillers=2)
            d1 = norm_thunks(1, sb1)
            while fillers:
                fillers.pop(0)()
                if d1:
                    d1.pop(0)()
            sb2 = attention_chunk(2, d1, use_fillers=0)
            sb3 = attention_chunk(3, norm_thunks(2, sb2), use_fillers=0)
            for t in norm_thunks(3, sb3):
                t()

    _split_multi_waits(nc)
    return nc


_CONSTS = _host_constants()
_PROGRAMS = {}


def _get_program(causal: bool):
    if causal not in _PROGRAMS:
        _PROGRAMS[causal] = _build_program(causal)
    return _PROGRAMS[causal]


def kernel(x, w_in, w_out, is_causal):
    causal = bool(np.asarray(is_causal).item())
    nc = _get_program(causal)

    x2 = np.asarray(x, dtype=np.float32).reshape(S, D)
    xT = np.ascontiguousarray(x2.T)                       # [D, S]
    w_in = np.asarray(w_in, dtype=np.float32)
    w_out = np.asarray(w_out, dtype=np.float32)

    in_maps = []
    for c in range(NCORES):
        r0 = c * EPC
        wq = w_in[r0:r0 + EPC, :]                          # [128, D]
        wk = w_in[D + r0:D + r0 + EPC, :]
        wv = w_in[2 * D + r0:2 * D + r0 + EPC, :]
        winT = np.ascontiguousarray(
            np.concatenate([wq, wk, wv], axis=0).T)        # [D, 384]
        woT0 = np.ascontiguousarray(w_out[:, r0:r0 + HD].T)        # [64, D]
        woT1 = np.ascontiguousarray(w_out[:, r0 + HD:r0 + EPC].T)  # [64, D]
        in_maps.append({"xT": xT, "winT": winT, "woT0": woT0, "woT1": woT1,
                        "rotT": _CONSTS[2], "tri": _CONSTS[3],
                        "ident": _CONSTS[4]})

    res = run_bass_kernel_spmd(nc, in_maps, list(range(NCORES)))
    out = np.zeros((S, D), dtype=np.float64)
    for c in range(NCORES):
        out += res.results[c]["pout"].astype(np.float64)
    return out.astype(np.float32).reshape(B, S, D)



# revision 8
# speedup vs baseline: 1.3876x; 1.3876x over previous
"""Tensor-parallel causal attention block (qkv proj + RoPE + attention + out proj)
for Trainium2, sharded over 8 NeuronCores by attention head (2 heads/core).

Contract: kernel(**inputs) takes the FULL inputs (x [1,2048,1024] f32,
w_in [3072,1024] f32, w_out [1024,1024] f32, is_causal scalar) and returns the
FULL output [1,2048,1024] f32.

Per-core layout strategy (everything kept transposed, [feature, seq], so no
on-device transposes of activations are ever needed):
  - host pre-transposes x -> xT [1024,2048] and the weight shards, casting all
    matmul operands to bf16 (full-rate on the PE, no fp32r power throttle)
  - qkvT = w_shard @ xT  ->  [384, 2048] (Q.T | K.T | V.T rows, 2 heads packed)
  - RoPE applied in [hd, s] layout via a constant rotation matmul + elementwise
  - scores computed transposed: S.T[k, q] = K @ Q.T  (softmax dim = partitions)
  - probs (unnormalized exp) hit PV directly:  ctx.T = V_aug.T @ P.T, where
    V_aug carries a ones column so row 64 of the PV output is the softmax
    denominator; normalization: PE broadcast of the denominator row to 64
    partitions, then a wide [64, QC] reciprocal + multiply on the DVE
  - out partial = ctxP @ w_outP (both heads packed, 128-deep contraction);
    partials stored fp16, host sums the 8 disjoint-head partials.
"""
import sys

sys.path.insert(0, '/opt/trn_rl_repo')

from contextlib import ExitStack

import numpy as np
import ml_dtypes

import concourse.bass as bass
from concourse import mybir, tile
from concourse.bass_utils import run_bass_kernel_spmd

B, S, D, H = 1, 2048, 1024, 16
HD = D // H            # 64
NCORES = 8
HPC = H // NCORES      # heads per core = 2
EPC = HPC * HD         # features per core = 128
ROPE_BASE = 10000.0

F32 = mybir.dt.float32
F32R = mybir.dt.float32r
BF16 = mybir.dt.bfloat16
FP16 = mybir.dt.float16
NPBF16 = ml_dtypes.bfloat16

QC = 512               # q-chunk width (one PSUM bank of fp32)
NQC = S // QC          # 4 q-chunks
NST = S // 128         # 16 s-tiles / k-tiles
ND = D // 128          # 8 contraction tiles for the input projection


def _split_multi_waits(nc, max_waits=1):
    """This container's walrus build accepts at most one embedded sync wait per
    instruction; move extra waits onto preceding same-engine NoOps."""
    n_split = 0
    for fn in nc.m.functions:
        for blk in fn.blocks:
            new_insts = []
            for inst in blk.instructions:
                si = inst.sync_info
                waits = list(si.on_wait) if (si and si.on_wait) else []
                if len(waits) > max_waits and inst.engine is not None:
                    for w in waits[max_waits:]:
                        nop = mybir.InstNoOp(
                            name=f"{inst.name}_wn{n_split}", ins=[], outs=[])
                        n_split += 1
                        nop.engine = inst.engine
                        nop.sync_info = mybir.SyncInfo(on_wait=[w], on_update=[])
                        nc.register_instruction(nop, overwrite=True)
                        new_insts.append(nop)
                    si.on_wait = waits[:max_waits]
                new_insts.append(inst)
            blk.instructions[:] = new_insts
    return n_split


def _host_constants():
    inv_freq = 1.0 / (ROPE_BASE ** (np.arange(0, HD, 2, dtype=np.float64) / HD))
    t = np.arange(S, dtype=np.float64)
    freqs = np.outer(inv_freq, t)                    # [32, S]  ([hd, s] layout)
    emb = np.concatenate([freqs, freqs], axis=0)     # [64, S]
    cosT = np.cos(emb).astype(NPBF16)
    sinT = np.sin(emb).astype(NPBF16)
    cos2 = np.tile(cosT, (2, 1))                     # [128, S] (2 heads packed)
    sin2 = np.tile(sinT, (2, 1))
    # rotate_half as a matrix: (R q)[i] = -q[i+32] (i<32), q[i-32] (i>=32)
    R = np.zeros((HD, HD), dtype=np.float32)
    for i in range(HD // 2):
        R[i, i + HD // 2] = -1.0
        R[i + HD // 2, i] = 1.0
    R2 = np.zeros((128, 128), dtype=np.float32)
    R2[0:64, 0:64] = R
    R2[64:128, 64:128] = R
    rotT = np.ascontiguousarray(R2.T).astype(NPBF16)
    # upper-triangular (k<=q) mask for the diagonal 128x128 blocks of S.T[k,q]
    tri = np.triu(np.ones((128, 128), dtype=np.float32)).astype(NPBF16)
    ident = np.eye(128, dtype=np.float32).astype(NPBF16)
    return cos2, sin2, rotT, tri, ident


def _build_program(causal: bool):
    nc = bass.Bass()
    xT_d = nc.dram_tensor("xT", [D, S], BF16, kind="ExternalInput")
    winT_d = nc.dram_tensor("winT", [D, 3 * EPC], BF16, kind="ExternalInput")
    woP_d = nc.dram_tensor("woP", [EPC, D], BF16, kind="ExternalInput")
    pout_d = nc.dram_tensor("pout", [S, D], FP16, kind="ExternalOutput")

    cos2_d = nc.dram_tensor("cos2", [128, S], BF16, kind="ExternalInput")
    sin2_d = nc.dram_tensor("sin2", [128, S], BF16, kind="ExternalInput")
    rotT_d = nc.dram_tensor("rotT", [128, 128], BF16, kind="ExternalInput")
    tri_d = nc.dram_tensor("tri", [128, 128], BF16, kind="ExternalInput")
    ident_d = nc.dram_tensor("ident", [128, 128], BF16, kind="ExternalInput")

    with tile.TileContext(nc) as tc, ExitStack() as ctx:
        sb = ctx.enter_context(tc.tile_pool(name="sb", bufs=1))
        wk0 = ctx.enter_context(tc.tile_pool(name="wk0", bufs=1))

        # ---- persistent SBUF tensors -----------------------------------
        winT = sb.tile([128, ND * 3 * EPC], BF16, name="winT")
        for d in range(ND):
            nc.sync.dma_start(winT[:, d * 3 * EPC:(d + 1) * 3 * EPC],
                              winT_d[d * 128:(d + 1) * 128, :])
        woP = sb.tile([EPC, D], BF16, name="woP")
        nc.sync.dma_start(woP[:], woP_d[:, :])
        cos2 = sb.tile([128, S], BF16, name="cos2")
        nc.sync.dma_start(cos2[:], cos2_d[:, :])
        sin2 = sb.tile([128, S], BF16, name="sin2")
        nc.sync.dma_start(sin2[:], sin2_d[:, :])
        rot = sb.tile([128, 128], BF16, name="rot")
        nc.sync.dma_start(rot[:], rotT_d[:, :])
        tri = sb.tile([128, 128], BF16, name="tri")
        nc.sync.dma_start(tri[:], tri_d[:, :])
        ident = sb.tile([128, 128], BF16, name="ident")
        nc.sync.dma_start(ident[:], ident_d[:, :])
        onesf = sb.tile([128, HD], F32R, name="onesf")
        nc.vector.memset(onesf[:].bitcast(F32), 1.0)

        qraw = sb.tile([128, S], BF16, name="qraw")
        kraw = sb.tile([128, S], BF16, name="kraw")
        vtr = sb.tile([128, S], BF16, name="vtr")
        qrot = sb.tile([128, S], BF16, name="qrot")
        krot = sb.tile([128, S], BF16, name="krot")
        vnat = sb.tile([128, NST * 130], BF16, name="vnat")
        nc.vector.memset(vnat[:], 1.0)
        heatout = sb.tile([1, 1], F32, name="heatout")
        ctxP = sb.tile([128, S], BF16, name="ctxP")

        def rope_chunk(c0, pfn):
            for (raw, out) in ((qraw, qrot), (kraw, krot)):
                rp = pfn()
                nc.tensor.matmul(rp[:, 0:QC], rot[:], raw[:, c0:c0 + QC],
                                 start=True, stop=True)
                t1 = wk0.tile([128, QC], BF16, tag="t1", bufs=3, name="t1")
                nc.vector.tensor_mul(t1[:], raw[:, c0:c0 + QC],
                                     cos2[:, c0:c0 + QC])
                t2 = wk0.tile([128, QC], BF16, tag="t2", bufs=3, name="t2")
                nc.vector.tensor_mul(t2[:], rp[:, 0:QC], sin2[:, c0:c0 + QC])
                nc.vector.tensor_add(out[:, c0:c0 + QC], t1[:], t2[:])

        def vt_tile(j, vfn):
            vp = vfn()
            nc.tensor.transpose(vp[:, 0:128], vtr[:, j * 128:(j + 1) * 128],
                                ident[:])
            nc.vector.tensor_copy(vnat[:, j * 130:j * 130 + 64], vp[:, 0:64])
            nc.vector.tensor_copy(vnat[:, j * 130 + 65:j * 130 + 129],
                                  vp[:, 64:128])

        # ========== Stage A, first half (s in [0, 1024)) ================
        with tc.tile_pool(name="psA", bufs=1, space="PSUM") as psA, \
             tc.tile_pool(name="wka", bufs=3) as wka:
            accs = [psA.tile([128, QC], F32, tag="acc", bufs=6, name="acc")
                    for _ in range(6)]
            for d in range(ND):
                xt = wka.tile([128, S // 2], BF16, tag="xt", name="xt")
                for half in range(2):
                    nc.sync.dma_start(
                        xt[:, half * QC:(half + 1) * QC],
                        xT_d[d * 128:(d + 1) * 128,
                             half * QC:(half + 1) * QC])
                for et in range(3):
                    lw = winT[:, d * 3 * EPC + et * 128:
                              d * 3 * EPC + (et + 1) * 128]
                    for sch in range(2):
                        nc.tensor.matmul(
                            accs[et * 2 + sch][:], lw,
                            xt[:, sch * QC:(sch + 1) * QC],
                            start=(d == 0), stop=(d == ND - 1))
            for sch in range(2):
                c0 = sch * QC
                nc.scalar.copy(qraw[:, c0:c0 + QC], accs[0 * 2 + sch][:])
                nc.scalar.copy(kraw[:, c0:c0 + QC], accs[1 * 2 + sch][:])
                nc.scalar.copy(vtr[:, c0:c0 + QC], accs[2 * 2 + sch][:])

            def pa_tile():
                return psA.tile([128, QC], F32, tag="acc", bufs=6, name="rp")

            def pa_vt():
                return psA.tile([128, 128], BF16, tag="acc", bufs=6,
                                name="vp")
            for sch in range(2):
                rope_chunk(sch * QC, pa_tile)
            for j in range(8):
                vt_tile(j, pa_vt)

        # ===== Stage B/C with stage-A-half-2 as filler thunks ===========
        with tc.tile_pool(name="psB", bufs=1, space="PSUM") as psB, \
             tc.tile_pool(name="wkb", bufs=3) as wkb:

            def op_tile():
                return psB.tile([128, QC], F32, tag="op", bufs=2, name="op")

            def op_vt():
                return psB.tile([128, 128], BF16, tag="op", bufs=2, name="vp")

            # issue all xT second-half loads now; they stream in during the
            # first attention chunks
            xts = {0: [], 1: []}
            for sch in range(2):
                for d in range(ND):
                    xt = wkb.tile([128, QC], BF16, tag="xt2", bufs=16,
                                  name="xt2")
                    c0 = S // 2 + sch * QC
                    nc.sync.dma_start(
                        xt[:], xT_d[d * 128:(d + 1) * 128, c0:c0 + QC])
                    xts[sch].append(xt)

            # ---- stage-A second-half work, chopped into filler thunks ----
            acc_h = {}

            def qkv_filler(sch, et, lohi):
                def f():
                    dlo, dhi = lohi
                    c0 = S // 2 + sch * QC
                    if dlo == 0:
                        acc_h[(sch, et)] = op_tile()
                    acc = acc_h[(sch, et)]
                    for d in range(dlo, dhi):
                        lw = winT[:, d * 3 * EPC + et * 128:
                                  d * 3 * EPC + (et + 1) * 128]
                        nc.tensor.matmul(acc[:], lw, xts[sch][d][:],
                                         start=(d == 0), stop=(d == ND - 1))
                    if dhi == ND:
                        dst = (qraw, kraw, vtr)[et]
                        nc.scalar.copy(dst[:, c0:c0 + QC], acc[:])
                        del acc_h[(sch, et)]
                return f

            fillers = []
            for sch in range(2):
                for et in range(3):
                    fillers.append(qkv_filler(sch, et, (0, 4)))
                    fillers.append(qkv_filler(sch, et, (4, ND)))
                    if et == 1:
                        c0 = S // 2 + sch * QC
                        fillers.append(
                            lambda c0=c0: rope_chunk(c0, op_tile))
                for j in range(8 + sch * 4, 12 + sch * 4):
                    fillers.append(lambda j=j: vt_tile(j, op_vt))

            def norm_thunks(qc, pvsb):
                thunks = []

                def norm(hh):
                    def f():
                        q0 = qc * QC
                        db = op_tile()
                        nc.tensor.matmul(db[0:HD, :], onesf[64:65, 0:HD],
                                         pvsb[hh][64:65, :],
                                         start=True, stop=True)
                        rcp = wkb.tile([HD, QC], F32, tag="rcp", bufs=4,
                                       name="rcp")
                        with nc.allow_low_precision(reason="denom recip"):
                            nc.vector.reciprocal(rcp[:], db[0:HD, :])
                        nc.vector.tensor_mul(
                            ctxP[hh * HD:(hh + 1) * HD, q0:q0 + QC],
                            pvsb[hh][0:64, :], rcp[:])
                    return f

                def oproj(sti):
                    def f():
                        c0 = (qc * 4 + sti) * 128
                        ob = wkb.tile([128, D], FP16, tag="ob", bufs=3,
                                      name="ob")
                        for dc in range(2):
                            op = op_tile()
                            nc.tensor.matmul(op[:], ctxP[:, c0:c0 + 128],
                                             woP[:, dc * QC:(dc + 1) * QC],
                                             start=True, stop=True)
                            if dc == 0:
                                nc.scalar.copy(ob[:, dc * QC:(dc + 1) * QC],
                                               op[:])
                            else:
                                nc.vector.tensor_copy(
                                    ob[:, dc * QC:(dc + 1) * QC], op[:])
                        nc.sync.dma_start(pout_d[c0:c0 + 128, :], ob[:])
                    return f

                thunks.append(norm(0))
                thunks.append(norm(1))
                for sti in range(4):
                    thunks.append(oproj(sti))
                return thunks

            LAG = 2

            def attention_chunk(qc, deferred, use_fillers):
                q0 = qc * QC
                n_k = 4 * (qc + 1) if causal else NST
                pvs = [psB.tile([65, QC], F32, tag="pv", bufs=2,
                                name=f"pv{hh}") for hh in range(2)]
                window = []

                def emit_pv(pkt, p0, last):
                    js = max(0, pkt - qc * 4) * 128 if causal else 0
                    for hh in range(2):
                        nc.tensor.matmul(
                            pvs[hh][:, js:QC],
                            vnat[:, pkt * 130 + hh * 65:
                                 pkt * 130 + hh * 65 + 65],
                            p0[:, hh * QC + js:hh * QC + QC],
                            start=(pkt == 0), stop=last)

                for kt in range(n_k):
                    st = psB.tile([128, 2 * QC], F32, tag="st", bufs=2,
                                  name="st")
                    for hh in range(2):
                        nc.tensor.matmul(
                            st[:, hh * QC:(hh + 1) * QC],
                            krot[hh * 64:(hh + 1) * 64,
                                 kt * 128:(kt + 1) * 128],
                            qrot[hh * 64:(hh + 1) * 64, q0:q0 + QC],
                            start=True, stop=True)
                    pt = wkb.tile([128, 2 * QC], BF16, tag="pt", bufs=4,
                                  name="pt")
                    j = kt - qc * 4
                    if causal and j >= 0:
                        for hh in range(2):
                            nc.scalar.activation(
                                pt[:, hh * QC + j * 128:(hh + 1) * QC],
                                st[:, hh * QC + j * 128:(hh + 1) * QC],
                                mybir.ActivationFunctionType.Exp, scale=0.125)
                            nc.vector.tensor_mul(
                                pt[:, hh * QC + j * 128:
                                   hh * QC + (j + 1) * 128],
                                pt[:, hh * QC + j * 128:
                                   hh * QC + (j + 1) * 128], tri[:])
                    else:
                        nc.scalar.activation(
                            pt[:], st[:],
                            mybir.ActivationFunctionType.Exp, scale=0.125)
                    window.append((kt, pt))
                    if len(window) > LAG:
                        emit_pv(*window.pop(0), last=False)
                    if deferred and kt >= 3 and kt % 2 == 1:
                        deferred.pop(0)()
                    if use_fillers and fillers:
                        fillers.pop(0)()
                        if use_fillers > 1 and fillers:
                            fillers.pop(0)()
                while window:
                    kt_, p_ = window.pop(0)
                    emit_pv(kt_, p_, last=(kt_ == n_k - 1))
                while deferred:
                    deferred.pop(0)()
                # evict PV accumulators to SBUF, freeing the PSUM banks
                pvsb = []
                for hh in range(2):
                    pb = wkb.tile([65, QC], F32R, tag="pvsb", bufs=4,
                                  name="pvsb")
                    nc.vector.tensor_copy(pb[:], pvs[hh][:])
                    pvsb.append(pb)
                return pvsb

            # warm-up burst bridging the rope->attention dependency gap
            heat = psB.tile([128, 2 * QC], F32, tag="st", bufs=2, name="heat")
            for _ in range(16):
                nc.tensor.matmul(heat[:, 0:QC], winT[:, 0:128],
                                 winT[:, 1024:1536], start=True, stop=True)
            nc.scalar.copy(heatout[:], heat[0:1, 0:1])

            sb0 = attention_chunk(0, [], use_fillers=0)
            sb1 = attention_chunk(1, norm_thunks(0, sb0), use_fillers=2)
            d1 = norm_thunks(1, sb1)
            while fillers:
                fillers.pop(0)()
                if d1:
                    d1.pop(0)()
            sb2 = attention_chunk(2, d1, use_fillers=0)
            sb3 = attention_chunk(3, norm_thunks(2, sb2), use_fillers=0)
            for t in norm_thunks(3, sb3):
                t()

    _split_multi_waits(nc)
    return nc


_CONSTS = _host_constants()
_PROGRAMS = {}


def _get_program(causal: bool):
    if causal not in _PROGRAMS:
        _PROGRAMS[causal] = _build_program(causal)
    return _PROGRAMS[causal]


def _make_in_maps(x, w_in, w_out):
    x2 = np.asarray(x, dtype=np.float32).reshape(S, D)
    xT = np.ascontiguousarray(x2.T).astype(NPBF16)        # [D, S]
    w_in = np.asarray(w_in, dtype=np.float32)
    w_out = np.asarray(w_out, dtype=np.float32)

    in_maps = []
    for c in range(NCORES):
        r0 = c * EPC
        wq = w_in[r0:r0 + EPC, :]                          # [128, D]
        wk = w_in[D + r0:D + r0 + EPC, :]
        wv = w_in[2 * D + r0:2 * D + r0 + EPC, :]
        winT = np.ascontiguousarray(
            np.concatenate([wq, wk, wv], axis=0).T).astype(NPBF16)  # [D, 384]
        woP = np.ascontiguousarray(
            w_out[:, r0:r0 + EPC].T).astype(NPBF16)        # [128, D]
        in_maps.append({"xT": xT, "winT": winT, "woP": woP,
                        "cos2": _CONSTS[0], "sin2": _CONSTS[1],
                        "rotT": _CONSTS[2], "tri": _CONSTS[3],
                        "ident": _CONSTS[4]})
    return in_maps


def kernel(x, w_in, w_out, is_causal):
    causal = bool(np.asarray(is_causal).item())
    nc = _get_program(causal)
    in_maps = _make_in_maps(x, w_in, w_out)
    res = run_bass_kernel_spmd(nc, in_maps, list(range(NCORES)))
    out = np.zeros((S, D), dtype=np.float64)
    for c in range(NCORES):
        out += res.results[c]["pout"].astype(np.float64)
    return out.astype(np.float32).reshape(B, S, D)


# revision 15
# speedup vs baseline: 1.6995x; 1.2248x over previous
"""Tensor-parallel causal attention block (qkv proj + RoPE + attention + out proj)
for Trainium2, sharded over 8 NeuronCores by attention head (2 heads/core).

Contract: kernel(**inputs) takes the FULL inputs (x [1,2048,1024] f32,
w_in [3072,1024] f32, w_out [1024,1024] f32, is_causal scalar) and returns the
FULL output [1,2048,1024] f32.

Per-core layout strategy (everything kept transposed, [feature, seq], so no
on-device transposes of activations are ever needed):
  - host pre-packs every input into a [128, N] partition-major image so each
    tensor is ONE contiguous dma (x is split into a handful of chunks so the
    first qkv matmul can start as soon as the first chunk lands); all matmul
    operands are bf16 (full PE rate, no fp32r power throttle)
  - a short warm-up burst of tiny matmuls runs while the first DMAs land,
    promoting the PE out of its low-power p-state before real work arrives
  - qkvT = w_shard @ xT  ->  [384, 2048] (Q.T | K.T | V.T rows, 2 heads packed)
  - RoPE applied in [hd, s] layout via a constant rotation matmul + elementwise;
    attention chunk 0 starts right after RoPE chunk 0, the rest of RoPE and the
    V transposes ride along as filler thunks inside chunk 0
  - scores computed transposed: S.T[k, q] = K @ Q.T (softmax dim = partitions),
    with the causally-dead q-range of diagonal k-tiles skipped
  - probs (unnormalized exp) hit PV directly:  ctx.T = V_aug.T @ P.T, where
    V_aug carries ones columns on each side of the 128 packed V features so
    PV also emits the per-q softmax denominators; normalization: PE broadcast
    of the denominator row to 64 partitions, then reciprocal_approx_fast + mul
  - out partial = ctxP @ w_outP (both heads packed, 128-deep contraction);
    partials stored fp16, host sums the 8 disjoint-head partials.
"""
import sys

sys.path.insert(0, '/opt/trn_rl_repo')

from contextlib import ExitStack

import numpy as np
import ml_dtypes

import concourse.bass as bass
from concourse import mybir, tile
from concourse.bass_utils import run_bass_kernel_spmd

B, S, D, H = 1, 2048, 1024, 16
HD = D // H            # 64
NCORES = 8
HPC = H // NCORES      # heads per core = 2
EPC = HPC * HD         # features per core = 128
ROPE_BASE = 10000.0

F32 = mybir.dt.float32
F32R = mybir.dt.float32r
BF16 = mybir.dt.bfloat16
FP16 = mybir.dt.float16
NPBF16 = ml_dtypes.bfloat16

QC = 512               # q-chunk width (one PSUM bank of fp32)
NQC = S // QC          # 4 q-chunks
NST = S // 128         # 16 s-tiles / k-tiles
ND = D // 128          # 8 contraction tiles for the input projection
VB = 130               # vnat block width: [V0 64 | ones | V1 64 | ones]


def _split_multi_waits(nc, max_waits=1):
    """This container's walrus build accepts at most one embedded sync wait per
    instruction; move extra waits onto preceding same-engine NoOps."""
    n_split = 0
    for fn in nc.m.functions:
        for blk in fn.blocks:
            new_insts = []
            for inst in blk.instructions:
                si = inst.sync_info
                waits = list(si.on_wait) if (si and si.on_wait) else []
                if len(waits) > max_waits and inst.engine is not None:
                    for w in waits[max_waits:]:
                        nop = mybir.InstNoOp(
                            name=f"{inst.name}_wn{n_split}", ins=[], outs=[])
                        n_split += 1
                        nop.engine = inst.engine
                        nop.sync_info = mybir.SyncInfo(on_wait=[w], on_update=[])
                        nc.register_instruction(nop, overwrite=True)
                        new_insts.append(nop)
                    si.on_wait = waits[:max_waits]
                new_insts.append(inst)
            blk.instructions[:] = new_insts
    return n_split


def _host_constants():
    inv_freq = 1.0 / (ROPE_BASE ** (np.arange(0, HD, 2, dtype=np.float64) / HD))
    t = np.arange(S, dtype=np.float64)
    freqs = np.outer(inv_freq, t)                    # [32, S]  ([hd, s] layout)
    emb = np.concatenate([freqs, freqs], axis=0)     # [64, S]
    cosT = np.cos(emb).astype(NPBF16)
    sinT = np.sin(emb).astype(NPBF16)
    cos2 = np.ascontiguousarray(np.tile(cosT, (2, 1)))  # [128, S] 2 heads
    sin2 = np.ascontiguousarray(np.tile(sinT, (2, 1)))
    # rotate_half as a matrix: (R q)[i] = -q[i+32] (i<32), q[i-32] (i>=32)
    R = np.zeros((HD, HD), dtype=np.float32)
    for i in range(HD // 2):
        R[i, i + HD // 2] = -1.0
        R[i + HD // 2, i] = 1.0
    R2 = np.zeros((128, 128), dtype=np.float32)
    R2[0:64, 0:64] = R
    R2[64:128, 64:128] = R
    rotT = np.ascontiguousarray(R2.T)
    # upper-triangular (k<=q) mask for the diagonal 128x128 blocks of S.T[k,q]
    tri = np.triu(np.ones((128, 128), dtype=np.float32))
    ident = np.eye(128, dtype=np.float32)
    # one packed [128, 384] image: rot | tri | ident
    consts = np.concatenate([rotT, tri, ident], axis=1).astype(NPBF16)
    return cos2, sin2, consts


def _build_program(causal: bool):
    nc = bass.Bass()
    # host-packed images, one DMA each (x in chunks)
    xA_d = nc.dram_tensor("xA", [128, ND * 1024], BF16, kind="ExternalInput")
    xB_d = nc.dram_tensor("xB", [128, ND * 1024], BF16, kind="ExternalInput")
    winP_d = nc.dram_tensor("winP", [128, ND * 3 * EPC], BF16,
                            kind="ExternalInput")
    woP_d = nc.dram_tensor("woP", [EPC, D], BF16, kind="ExternalInput")
    cos2_d = nc.dram_tensor("cos2", [128, S], BF16, kind="ExternalInput")
    sin2_d = nc.dram_tensor("sin2", [128, S], BF16, kind="ExternalInput")
    consts_d = nc.dram_tensor("consts", [128, 384], BF16, kind="ExternalInput")
    pout_d = nc.dram_tensor("pout", [S, D], FP16, kind="ExternalOutput")

    with tile.TileContext(nc) as tc, ExitStack() as ctx:
        sb = ctx.enter_context(tc.tile_pool(name="sb", bufs=1))
        wk0 = ctx.enter_context(tc.tile_pool(name="wk0", bufs=1))

        # ---- persistent SBUF tensors -----------------------------------
        winT = sb.tile([128, ND * 3 * EPC], BF16, name="winT")
        xA = sb.tile([128, ND * 1024], BF16, name="xA")
        xB = sb.tile([128, ND * 1024], BF16, name="xB")
        consts = sb.tile([128, 384], BF16, name="consts")
        cos2 = sb.tile([128, S], BF16, name="cos2")
        sin2 = sb.tile([128, S], BF16, name="sin2")
        woP = sb.tile([EPC, D], BF16, name="woP")
        rot = consts[:, 0:128]
        tri = consts[:, 128:256]
        ident = consts[:, 256:384]

        # sync queue: weights first, then x chunks (first qkv matmul needs
        # winT half 1 + xA chunk 0 only)
        nc.sync.dma_start(winT[:, 0:1536], winP_d[:, 0:1536])
        nc.sync.dma_start(xA[:, 0:2048], xA_d[:, 0:2048])
        nc.sync.dma_start(winT[:, 1536:3072], winP_d[:, 1536:3072])
        for c in range(1, 4):
            nc.sync.dma_start(xA[:, c * 2048:(c + 1) * 2048],
                              xA_d[:, c * 2048:(c + 1) * 2048])
        # gpsimd queue: rope constants (needed ~15us in)
        nc.gpsimd.dma_start(consts[:], consts_d[:, :])
        nc.gpsimd.dma_start(cos2[:], cos2_d[:, :])
        nc.gpsimd.dma_start(sin2[:], sin2_d[:, :])
        # scalar queue: second-half x and the output weights (needed late)
        nc.scalar.dma_start(xB[:], xB_d[:, :])
        nc.scalar.dma_start(woP[:], woP_d[:, :])

        onesf = sb.tile([128, HD], F32R, name="onesf")
        nc.vector.memset(onesf[:].bitcast(F32), 1.0)

        qraw = sb.tile([128, S], BF16, name="qraw")
        kraw = sb.tile([128, S], BF16, name="kraw")
        vtr = sb.tile([128, S], BF16, name="vtr")
        qrot = sb.tile([128, S], BF16, name="qrot")
        krot = sb.tile([128, S], BF16, name="krot")
        vnat = sb.tile([128, NST * VB], BF16, name="vnat")
        nc.vector.memset(vnat[:], 1.0)
        heatout = sb.tile([1, 1], F32, name="heatout")
        ctxP = sb.tile([128, S], BF16, name="ctxP")

        def rope_chunk(c0, pfn):
            for (raw, out) in ((qraw, qrot), (kraw, krot)):
                rp = pfn()
                nc.tensor.matmul(rp[:, 0:QC], rot, raw[:, c0:c0 + QC],
                                 start=True, stop=True)
                t1 = wk0.tile([128, QC], BF16, tag="t1", bufs=3, name="t1")
                nc.vector.tensor_mul(t1[:], raw[:, c0:c0 + QC],
                                     cos2[:, c0:c0 + QC])
                t2 = wk0.tile([128, QC], BF16, tag="t2", bufs=3, name="t2")
                nc.vector.tensor_mul(t2[:], rp[:, 0:QC], sin2[:, c0:c0 + QC])
                nc.vector.tensor_add(out[:, c0:c0 + QC], t1[:], t2[:])

        def vt_tile(j, vfn):
            vp = vfn()
            nc.tensor.transpose(vp[:, 0:128], vtr[:, j * 128:(j + 1) * 128],
                                ident)
            nc.vector.tensor_copy(vnat[:, j * VB:j * VB + 64], vp[:, 0:64])
            nc.vector.tensor_copy(vnat[:, j * VB + 65:j * VB + 129],
                                  vp[:, 64:128])

        # ========== Stage A, first half (s in [0, 1024)) ================
        with tc.tile_pool(name="psA", bufs=1, space="PSUM") as psA:
            # p-state warm-up on junk data while the first DMAs land
            heat = psA.tile([HD, HD], F32, tag="acc", bufs=6, name="heat")
            for _ in range(20):
                nc.tensor.matmul(heat[:], onesf[:, 0:HD], onesf[:, 0:HD],
                                 start=True, stop=True)
            nc.scalar.copy(heatout[:], heat[0:1, 0:1])

            accs = [psA.tile([128, QC], F32, tag="acc", bufs=6, name="acc")
                    for _ in range(6)]
            for d in range(ND):
                for et in range(3):
                    lw = winT[:, d * 3 * EPC + et * 128:
                              d * 3 * EPC + (et + 1) * 128]
                    for sch in range(2):
                        nc.tensor.matmul(
                            accs[et * 2 + sch][:], lw,
                            xA[:, d * 1024 + sch * QC:
                               d * 1024 + (sch + 1) * QC],
                            start=(d == 0), stop=(d == ND - 1))
            for sch in range(2):
                c0 = sch * QC
                nc.scalar.copy(qraw[:, c0:c0 + QC], accs[0 * 2 + sch][:])
                nc.scalar.copy(kraw[:, c0:c0 + QC], accs[1 * 2 + sch][:])
                nc.scalar.copy(vtr[:, c0:c0 + QC], accs[2 * 2 + sch][:])

            def pa_tile():
                return psA.tile([128, QC], F32, tag="acc", bufs=6, name="rp")

            def pa_vt():
                return psA.tile([128, 128], BF16, tag="acc", bufs=6,
                                name="vp")
            rope_chunk(0, pa_tile)
            for j in range(4):
                vt_tile(j, pa_vt)

        # ===== Stage B/C: attention with deferred/filler interleaving ===
        with tc.tile_pool(name="psB", bufs=1, space="PSUM") as psB, \
             tc.tile_pool(name="wkb", bufs=3) as wkb:

            def op_tile():
                return psB.tile([128, QC], F32, tag="op", bufs=2, name="op")

            def op_vt():
                return psB.tile([128, 128], BF16, tag="op", bufs=2, name="vp")

            # rest of rope + V transposes ride inside attention chunk 0
            fillers0 = [lambda: rope_chunk(QC, op_tile)]
            for j in range(4, 8):
                fillers0.append(lambda j=j: vt_tile(j, op_vt))

            # ---- stage-A second-half work, chopped into filler thunks ----
            acc_h = {}

            def qkv_filler(sch, et, lohi):
                def f():
                    dlo, dhi = lohi
                    c0 = S // 2 + sch * QC
                    if dlo == 0:
                        acc_h[(sch, et)] = op_tile()
                    acc = acc_h[(sch, et)]
                    for d in range(dlo, dhi):
                        lw = winT[:, d * 3 * EPC + et * 128:
                                  d * 3 * EPC + (et + 1) * 128]
                        nc.tensor.matmul(
                            acc[:], lw,
                            xB[:, d * 1024 + sch * QC:
                               d * 1024 + (sch + 1) * QC],
                            start=(d == 0), stop=(d == ND - 1))
                    if dhi == ND:
                        dst = (qraw, kraw, vtr)[et]
                        nc.vector.tensor_copy(dst[:, c0:c0 + QC], acc[:])
                        del acc_h[(sch, et)]
                return f

            fillers = []
            for sch in range(2):
                for et in range(3):
                    fillers.append(qkv_filler(sch, et, (0, 4)))
                    fillers.append(qkv_filler(sch, et, (4, ND)))
                    if et == 1:
                        c0 = S // 2 + sch * QC
                        fillers.append(
                            lambda c0=c0: rope_chunk(c0, op_tile))
                for j in range(8 + sch * 4, 12 + sch * 4):
                    fillers.append(lambda j=j: vt_tile(j, op_vt))

            def norm_thunks(qc, pvsb):
                thunks = []

                def norm(hh):
                    def f():
                        q0 = qc * QC
                        db = op_tile()
                        nc.tensor.matmul(db[0:HD, :], onesf[64:65, 0:HD],
                                         pvsb[hh][64:65, :],
                                         start=True, stop=True)
                        rcp = wkb.tile([HD, QC], F32, tag="rcp", bufs=4,
                                       name="rcp")
                        # 1/d = exp(-ln d); ln+exp share one ACT table set
                        nc.scalar.activation(rcp[:], db[0:HD, :],
                                             mybir.ActivationFunctionType.Ln)
                        nc.scalar.activation(rcp[:], rcp[:],
                                             mybir.ActivationFunctionType.Exp,
                                             scale=-1.0)
                        nc.vector.tensor_mul(
                            ctxP[hh * HD:(hh + 1) * HD, q0:q0 + QC],
                            pvsb[hh][0:64, :], rcp[:])
                    return f

                def oproj(sti):
                    def f():
                        c0 = (qc * 4 + sti) * 128
                        ob = wkb.tile([128, D], FP16, tag="ob", bufs=3,
                                      name="ob")
                        for dc in range(2):
                            op = op_tile()
                            nc.tensor.matmul(op[:], ctxP[:, c0:c0 + 128],
                                             woP[:, dc * QC:(dc + 1) * QC],
                                             start=True, stop=True)
                            if dc == 0:
                                nc.scalar.copy(ob[:, dc * QC:(dc + 1) * QC],
                                               op[:])
                            else:
                                nc.vector.tensor_copy(
                                    ob[:, dc * QC:(dc + 1) * QC], op[:])
                        nc.sync.dma_start(pout_d[c0:c0 + 128, :], ob[:])
                    return f

                thunks.append(norm(0))
                thunks.append(norm(1))
                for sti in range(4):
                    thunks.append(oproj(sti))
                return thunks

            LAG = 2

            def attention_chunk(qc, deferred, fill, per_kt=1):
                q0 = qc * QC
                n_k = 4 * (qc + 1) if causal else NST
                pvs = [psB.tile([65, QC], F32, tag="pv", bufs=2,
                                name=f"pv{hh}") for hh in range(2)]
                window = []

                def emit_pv(pkt, p0, last):
                    js = max(0, pkt - qc * 4) * 128 if causal else 0
                    for hh in range(2):
                        nc.tensor.matmul(
                            pvs[hh][:, js:QC],
                            vnat[:, pkt * VB + hh * 65:
                                 pkt * VB + hh * 65 + 65],
                            p0[:, hh * QC + js:hh * QC + QC],
                            start=(pkt == 0), stop=last)

                for kt in range(n_k):
                    j = kt - qc * 4
                    js = max(0, j) * 128 if causal else 0
                    st = psB.tile([128, 2 * QC], F32, tag="st", bufs=2,
                                  name="st")
                    for hh in range(2):
                        nc.tensor.matmul(
                            st[:, hh * QC + js:(hh + 1) * QC],
                            krot[hh * 64:(hh + 1) * 64,
                                 kt * 128:(kt + 1) * 128],
                            qrot[hh * 64:(hh + 1) * 64, q0 + js:q0 + QC],
                            start=True, stop=True)
                    pt = wkb.tile([128, 2 * QC], BF16, tag="pt", bufs=4,
                                  name="pt")
                    if causal and j >= 0:
                        for hh in range(2):
                            nc.scalar.activation(
                                pt[:, hh * QC + j * 128:(hh + 1) * QC],
                                st[:, hh * QC + j * 128:(hh + 1) * QC],
                                mybir.ActivationFunctionType.Exp, scale=0.125)
                            nc.vector.tensor_mul(
                                pt[:, hh * QC + j * 128:
                                   hh * QC + (j + 1) * 128],
                                pt[:, hh * QC + j * 128:
                                   hh * QC + (j + 1) * 128], tri)
                    else:
                        nc.scalar.activation(
                            pt[:], st[:],
                            mybir.ActivationFunctionType.Exp, scale=0.125)
                    window.append((kt, pt))
                    if len(window) > LAG:
                        emit_pv(*window.pop(0), last=False)
                    if deferred and kt >= 3 and kt % 2 == 1:
                        deferred.pop(0)()
                    if fill:
                        for _ in range(per_kt):
                            if fill:
                                fill.pop(0)()
                while window:
                    kt_, p_ = window.pop(0)
                    emit_pv(kt_, p_, last=(kt_ == n_k - 1))
                while deferred:
                    deferred.pop(0)()
                # evict PV accumulators to SBUF, freeing the PSUM banks
                pvsb = []
                for hh in range(2):
                    pb = wkb.tile([65, QC], F32R, tag="pvsb", bufs=4,
                                  name="pvsb")
                    nc.vector.tensor_copy(pb[:], pvs[hh][:])
                    pvsb.append(pb)
                return pvsb

            sb0 = attention_chunk(0, [], fillers0, per_kt=2)
            sb1 = attention_chunk(1, norm_thunks(0, sb0), fillers, per_kt=2)
            d1 = norm_thunks(1, sb1)
            while fillers:
                fillers.pop(0)()
                if d1:
                    d1.pop(0)()
            sb2 = attention_chunk(2, d1, [])
            sb3 = attention_chunk(3, norm_thunks(2, sb2), [])
            for t in norm_thunks(3, sb3):
                t()

    _split_multi_waits(nc)
    return nc


_CONSTS = _host_constants()
_PROGRAMS = {}


def _get_program(causal: bool):
    if causal not in _PROGRAMS:
        _PROGRAMS[causal] = _build_program(causal)
    return _PROGRAMS[causal]


def _make_in_maps(x, w_in, w_out):
    x2 = np.asarray(x, dtype=np.float32).reshape(S, D)
    xT = np.ascontiguousarray(x2.T).astype(NPBF16)        # [D, S]
    # pack: xh[p, half*8192 + d*1024 + sl] = xT[d*128+p, half*1024+sl]
    xq = xT.reshape(ND, 128, 2, 1024).transpose(2, 1, 0, 3)  # [half,p,d,sl]
    xAh = np.ascontiguousarray(xq[0].reshape(128, ND * 1024))
    xBh = np.ascontiguousarray(xq[1].reshape(128, ND * 1024))
    w_in = np.asarray(w_in, dtype=np.float32)
    w_out = np.asarray(w_out, dtype=np.float32)

    in_maps = []
    for c in range(NCORES):
        r0 = c * EPC
        wq = w_in[r0:r0 + EPC, :]                          # [128, D]
        wk = w_in[D + r0:D + r0 + EPC, :]
        wv = w_in[2 * D + r0:2 * D + r0 + EPC, :]
        winT = np.ascontiguousarray(
            np.concatenate([wq, wk, wv], axis=0).T)        # [D, 384]
        winP = np.ascontiguousarray(
            winT.reshape(ND, 128, 3 * EPC).transpose(1, 0, 2)
            .reshape(128, ND * 3 * EPC)).astype(NPBF16)
        woP = np.ascontiguousarray(
            w_out[:, r0:r0 + EPC].T).astype(NPBF16)        # [128, D]
        in_maps.append({"xA": xAh, "xB": xBh, "winP": winP, "woP": woP,
                        "cos2": _CONSTS[0], "sin2": _CONSTS[1],
                        "consts": _CONSTS[2]})
    return in_maps


def kernel(x, w_in, w_out, is_causal):
    causal = bool(np.asarray(is_causal).item())
    nc = _get_program(causal)
    in_maps = _make_in_maps(x, w_in, w_out)
    res = run_bass_kernel_spmd(nc, in_maps, list(range(NCORES)))
    out = np.zeros((S, D), dtype=np.float64)
    for c in range(NCORES):
        out += res.results[c]["pout"].astype(np.float64)
    return out.astype(np.float32).reshape(B, S, D)


# revision 18
# speedup vs baseline: 1.7534x; 1.0317x over previous
"""Tensor-parallel causal attention block (qkv proj + RoPE + attention + out proj)
for Trainium2, sharded over 8 NeuronCores by attention head (2 heads/core).

Contract: kernel(**inputs) takes the FULL inputs (x [1,2048,1024] f32,
w_in [3072,1024] f32, w_out [1024,1024] f32, is_causal scalar) and returns the
FULL output [1,2048,1024] f32.

Per-core layout strategy (everything kept transposed, [feature, seq], so no
on-device transposes of activations are ever needed):
  - host pre-packs every input into a [128, N] partition-major image so each
    tensor is ONE contiguous dma (x is split into a handful of chunks so the
    first qkv matmul can start as soon as the first chunk lands); all matmul
    operands are bf16 (full PE rate, no fp32r power throttle)
  - a short warm-up burst of tiny matmuls runs while the first DMAs land,
    promoting the PE out of its low-power p-state before real work arrives
  - qkvT = w_shard @ xT  ->  [384, 2048] (Q.T | K.T | V.T rows, 2 heads packed)
  - RoPE applied in [hd, s] layout via a constant rotation matmul + elementwise;
    attention chunk 0 starts right after RoPE chunk 0, the rest of RoPE and the
    V transposes ride along as filler thunks inside chunk 0
  - scores computed transposed: S.T[k, q] = K @ Q.T (softmax dim = partitions),
    with the causally-dead q-range of diagonal k-tiles skipped
  - probs (unnormalized exp) hit PV directly:  ctx.T = V_aug.T @ P.T, where
    V_aug carries ones columns on each side of the 128 packed V features so
    PV also emits the per-q softmax denominators; normalization: PE broadcast
    of the denominator row to 64 partitions, then reciprocal_approx_fast + mul
  - out partial = ctxP @ w_outP (both heads packed, 128-deep contraction);
    partials stored fp16, host sums the 8 disjoint-head partials.
"""
import sys

sys.path.insert(0, '/opt/trn_rl_repo')

from contextlib import ExitStack

import numpy as np
import ml_dtypes

import concourse.bass as bass
from concourse import mybir, tile
from concourse.bass_utils import run_bass_kernel_spmd

B, S, D, H = 1, 2048, 1024, 16
HD = D // H            # 64
NCORES = 8
HPC = H // NCORES      # heads per core = 2
EPC = HPC * HD         # features per core = 128
ROPE_BASE = 10000.0

F32 = mybir.dt.float32
F32R = mybir.dt.float32r
BF16 = mybir.dt.bfloat16
FP16 = mybir.dt.float16
NPBF16 = ml_dtypes.bfloat16

QC = 512               # q-chunk width (one PSUM bank of fp32)
NQC = S // QC          # 4 q-chunks
NST = S // 128         # 16 s-tiles / k-tiles
ND = D // 128          # 8 contraction tiles for the input projection
VB = 130               # vnat block width: [V0 64 | ones | V1 64 | ones]


def _split_multi_waits(nc, max_waits=1):
    """This container's walrus build accepts at most one embedded sync wait per
    instruction; move extra waits onto preceding same-engine NoOps."""
    n_split = 0
    for fn in nc.m.functions:
        for blk in fn.blocks:
            new_insts = []
            for inst in blk.instructions:
                si = inst.sync_info
                waits = list(si.on_wait) if (si and si.on_wait) else []
                if len(waits) > max_waits and inst.engine is not None:
                    for w in waits[max_waits:]:
                        nop = mybir.InstNoOp(
                            name=f"{inst.name}_wn{n_split}", ins=[], outs=[])
                        n_split += 1
                        nop.engine = inst.engine
                        nop.sync_info = mybir.SyncInfo(on_wait=[w], on_update=[])
                        nc.register_instruction(nop, overwrite=True)
                        new_insts.append(nop)
                    si.on_wait = waits[:max_waits]
                new_insts.append(inst)
            blk.instructions[:] = new_insts
    return n_split


def _host_constants():
    inv_freq = 1.0 / (ROPE_BASE ** (np.arange(0, HD, 2, dtype=np.float64) / HD))
    t = np.arange(S, dtype=np.float64)
    freqs = np.outer(inv_freq, t)                    # [32, S]  ([hd, s] layout)
    emb = np.concatenate([freqs, freqs], axis=0)     # [64, S]
    cosT = np.cos(emb).astype(NPBF16)
    sinT = np.sin(emb).astype(NPBF16)
    cos2 = np.ascontiguousarray(np.tile(cosT, (2, 1)))  # [128, S] 2 heads
    sin2 = np.ascontiguousarray(np.tile(sinT, (2, 1)))
    # rotate_half as a matrix: (R q)[i] = -q[i+32] (i<32), q[i-32] (i>=32)
    R = np.zeros((HD, HD), dtype=np.float32)
    for i in range(HD // 2):
        R[i, i + HD // 2] = -1.0
        R[i + HD // 2, i] = 1.0
    R2 = np.zeros((128, 128), dtype=np.float32)
    R2[0:64, 0:64] = R
    R2[64:128, 64:128] = R
    rotT = np.ascontiguousarray(R2.T)
    # upper-triangular (k<=q) mask for the diagonal 128x128 blocks of S.T[k,q]
    tri = np.triu(np.ones((128, 128), dtype=np.float32))
    ident = np.eye(128, dtype=np.float32)
    # one packed [128, 384] image: rot | tri | ident
    consts = np.concatenate([rotT, tri, ident], axis=1).astype(NPBF16)
    return cos2, sin2, consts


def _build_program(causal: bool):
    nc = bass.Bass()
    # host-packed images, one DMA each (x in chunks)
    xA_d = nc.dram_tensor("xA", [128, ND * 1024], BF16, kind="ExternalInput")
    xB_d = nc.dram_tensor("xB", [128, ND * 1024], BF16, kind="ExternalInput")
    winP_d = nc.dram_tensor("winP", [128, ND * 3 * EPC], BF16,
                            kind="ExternalInput")
    woP_d = nc.dram_tensor("woP", [EPC, D], BF16, kind="ExternalInput")
    cos2_d = nc.dram_tensor("cos2", [128, S], BF16, kind="ExternalInput")
    sin2_d = nc.dram_tensor("sin2", [128, S], BF16, kind="ExternalInput")
    consts_d = nc.dram_tensor("consts", [128, 384], BF16, kind="ExternalInput")
    pout_d = nc.dram_tensor("pout", [S, D], FP16, kind="ExternalOutput")

    with tile.TileContext(nc) as tc, ExitStack() as ctx:
        sb = ctx.enter_context(tc.tile_pool(name="sb", bufs=1))
        wk0 = ctx.enter_context(tc.tile_pool(name="wk0", bufs=1))

        # ---- persistent SBUF tensors -----------------------------------
        winT = sb.tile([128, ND * 3 * EPC], BF16, name="winT")
        xA = sb.tile([128, ND * 1024], BF16, name="xA")
        xB = sb.tile([128, ND * 1024], BF16, name="xB")
        consts = sb.tile([128, 384], BF16, name="consts")
        cos2 = sb.tile([128, S], BF16, name="cos2")
        sin2 = sb.tile([128, S], BF16, name="sin2")
        woP = sb.tile([EPC, D], BF16, name="woP")
        rot = consts[:, 0:128]
        tri = consts[:, 128:256]
        ident = consts[:, 256:384]

        # sync queue carries ONLY what the first qkv matmuls need — the DMA
        # rings interleave all queued transfers, so anything else on this
        # queue delays the critical first chunk
        nc.sync.dma_start(winT[:, 0:1536], winP_d[:, 0:1536])
        nc.sync.dma_start(xA[:, 0:2048], xA_d[:, 0:2048])
        # scalar queue: the rest of stage A (needed from ~25us)
        nc.scalar.dma_start(winT[:, 1536:3072], winP_d[:, 1536:3072])
        for c in range(1, 4):
            nc.scalar.dma_start(xA[:, c * 2048:(c + 1) * 2048],
                                xA_d[:, c * 2048:(c + 1) * 2048])
        # gpsimd queue: rope constants, then late-needed tensors
        nc.gpsimd.dma_start(consts[:], consts_d[:, :])
        nc.gpsimd.dma_start(cos2[:], cos2_d[:, :])
        nc.gpsimd.dma_start(sin2[:], sin2_d[:, :])
        nc.gpsimd.dma_start(xB[:], xB_d[:, :])
        nc.gpsimd.dma_start(woP[:], woP_d[:, :])

        onesf = sb.tile([128, HD], F32R, name="onesf")
        nc.vector.memset(onesf[:].bitcast(F32), 1.0)

        qraw = sb.tile([128, S], BF16, name="qraw")
        kraw = sb.tile([128, S], BF16, name="kraw")
        vtr = sb.tile([128, S], BF16, name="vtr")
        qrot = sb.tile([128, S], BF16, name="qrot")
        krot = sb.tile([128, S], BF16, name="krot")
        vnat = sb.tile([128, NST * VB], BF16, name="vnat")
        nc.vector.memset(vnat[:], 1.0)
        heatout = sb.tile([1, 1], F32, name="heatout")
        ctxP = sb.tile([128, S], BF16, name="ctxP")

        def rope_chunk(c0, pfn):
            for (raw, out) in ((qraw, qrot), (kraw, krot)):
                rp = pfn()
                nc.tensor.matmul(rp[:, 0:QC], rot, raw[:, c0:c0 + QC],
                                 start=True, stop=True)
                t1 = wk0.tile([128, QC], BF16, tag="t1", bufs=3, name="t1")
                nc.vector.tensor_mul(t1[:], raw[:, c0:c0 + QC],
                                     cos2[:, c0:c0 + QC])
                t2 = wk0.tile([128, QC], BF16, tag="t2", bufs=3, name="t2")
                nc.vector.tensor_mul(t2[:], rp[:, 0:QC], sin2[:, c0:c0 + QC])
                nc.vector.tensor_add(out[:, c0:c0 + QC], t1[:], t2[:])

        def vt_tile(j, vfn):
            vp = vfn()
            nc.tensor.transpose(vp[:, 0:128], vtr[:, j * 128:(j + 1) * 128],
                                ident)
            nc.vector.tensor_copy(vnat[:, j * VB:j * VB + 64], vp[:, 0:64])
            nc.vector.tensor_copy(vnat[:, j * VB + 65:j * VB + 129],
                                  vp[:, 64:128])

        # ========== Stage A, first half (s in [0, 1024)) ================
        with tc.tile_pool(name="psA", bufs=1, space="PSUM") as psA:
            # p-state warm-up on junk data while the first DMAs land
            heat = psA.tile([HD, HD], F32, tag="acc", bufs=6, name="heat")
            for _ in range(44):
                nc.tensor.matmul(heat[:], onesf[:, 0:HD], onesf[:, 0:HD],
                                 start=True, stop=True)
            nc.scalar.copy(heatout[:], heat[0:1, 0:1])

            accs = [psA.tile([128, QC], F32, tag="acc", bufs=6, name="acc")
                    for _ in range(6)]
            for d in range(ND):
                for et in range(3):
                    lw = winT[:, d * 3 * EPC + et * 128:
                              d * 3 * EPC + (et + 1) * 128]
                    for sch in range(2):
                        nc.tensor.matmul(
                            accs[et * 2 + sch][:], lw,
                            xA[:, d * 1024 + sch * QC:
                               d * 1024 + (sch + 1) * QC],
                            start=(d == 0), stop=(d == ND - 1))
            for sch in range(2):
                c0 = sch * QC
                nc.scalar.copy(qraw[:, c0:c0 + QC], accs[0 * 2 + sch][:])
                nc.scalar.copy(kraw[:, c0:c0 + QC], accs[1 * 2 + sch][:])
                nc.scalar.copy(vtr[:, c0:c0 + QC], accs[2 * 2 + sch][:])

            def pa_tile():
                return psA.tile([128, QC], F32, tag="acc", bufs=6, name="rp")

            def pa_vt():
                return psA.tile([128, 128], BF16, tag="acc", bufs=6,
                                name="vp")
            rope_chunk(0, pa_tile)
            for j in range(4):
                vt_tile(j, pa_vt)

        # ===== Stage B/C: attention with deferred/filler interleaving ===
        with tc.tile_pool(name="psB", bufs=1, space="PSUM") as psB, \
             tc.tile_pool(name="wkb", bufs=3) as wkb:

            def op_tile():
                return psB.tile([128, QC], F32, tag="op", bufs=2, name="op")

            def op_vt():
                return psB.tile([128, 128], BF16, tag="op", bufs=2, name="vp")

            # rest of rope + V transposes ride inside attention chunk 0
            fillers0 = [lambda: rope_chunk(QC, op_tile)]
            for j in range(4, 8):
                fillers0.append(lambda j=j: vt_tile(j, op_vt))

            # ---- stage-A second-half work, chopped into filler thunks ----
            acc_h = {}

            def qkv_filler(sch, et, lohi):
                def f():
                    dlo, dhi = lohi
                    c0 = S // 2 + sch * QC
                    if dlo == 0:
                        acc_h[(sch, et)] = op_tile()
                    acc = acc_h[(sch, et)]
                    for d in range(dlo, dhi):
                        lw = winT[:, d * 3 * EPC + et * 128:
                                  d * 3 * EPC + (et + 1) * 128]
                        nc.tensor.matmul(
                            acc[:], lw,
                            xB[:, d * 1024 + sch * QC:
                               d * 1024 + (sch + 1) * QC],
                            start=(d == 0), stop=(d == ND - 1))
                    if dhi == ND:
                        dst = (qraw, kraw, vtr)[et]
                        nc.vector.tensor_copy(dst[:, c0:c0 + QC], acc[:])
                        del acc_h[(sch, et)]
                return f

            fillers = []
            for sch in range(2):
                for et in range(3):
                    fillers.append(qkv_filler(sch, et, (0, 4)))
                    fillers.append(qkv_filler(sch, et, (4, ND)))
                    if et == 1:
                        c0 = S // 2 + sch * QC
                        fillers.append(
                            lambda c0=c0: rope_chunk(c0, op_tile))
                for j in range(8 + sch * 4, 12 + sch * 4):
                    fillers.append(lambda j=j: vt_tile(j, op_vt))

            def norm_thunks(qc, pvsb):
                thunks = []

                def norm(hh):
                    def f():
                        q0 = qc * QC
                        db = op_tile()
                        nc.tensor.matmul(db[0:HD, :], onesf[64:65, 0:HD],
                                         pvsb[hh][64:65, :],
                                         start=True, stop=True)
                        rcp = wkb.tile([HD, QC], F32, tag="rcp", bufs=4,
                                       name="rcp")
                        # 1/d = exp(-ln d); ln+exp share one ACT table set
                        nc.scalar.activation(rcp[:], db[0:HD, :],
                                             mybir.ActivationFunctionType.Ln)
                        nc.scalar.activation(rcp[:], rcp[:],
                                             mybir.ActivationFunctionType.Exp,
                                             scale=-1.0)
                        nc.vector.tensor_mul(
                            ctxP[hh * HD:(hh + 1) * HD, q0:q0 + QC],
                            pvsb[hh][0:64, :], rcp[:])
                    return f

                def oproj(sti):
                    def f():
                        c0 = (qc * 4 + sti) * 128
                        ob = wkb.tile([128, D], FP16, tag="ob", bufs=3,
                                      name="ob")
                        for dc in range(2):
                            op = op_tile()
                            nc.tensor.matmul(op[:], ctxP[:, c0:c0 + 128],
                                             woP[:, dc * QC:(dc + 1) * QC],
                                             start=True, stop=True)
                            nc.vector.tensor_copy(
                                ob[:, dc * QC:(dc + 1) * QC], op[:])
                        nc.sync.dma_start(pout_d[c0:c0 + 128, :], ob[:])
                    return f

                thunks.append(norm(0))
                thunks.append(norm(1))
                for sti in range(4):
                    thunks.append(oproj(sti))
                return thunks

            LAG = 2

            def attention_chunk(qc, deferred, fill, per_kt=1):
                q0 = qc * QC
                n_k = 4 * (qc + 1) if causal else NST
                pvs = [psB.tile([65, QC], F32, tag="pv", bufs=2,
                                name=f"pv{hh}") for hh in range(2)]
                window = []

                def emit_pv(pkt, p0, last):
                    js = max(0, pkt - qc * 4) * 128 if causal else 0
                    for hh in range(2):
                        nc.tensor.matmul(
                            pvs[hh][:, js:QC],
                            vnat[:, pkt * VB + hh * 65:
                                 pkt * VB + hh * 65 + 65],
                            p0[:, hh * QC + js:hh * QC + QC],
                            start=(pkt == 0), stop=last)

                for kt in range(n_k):
                    j = kt - qc * 4
                    js = max(0, j) * 128 if causal else 0
                    st = psB.tile([128, 2 * QC], F32, tag="st", bufs=2,
                                  name="st")
                    for hh in range(2):
                        nc.tensor.matmul(
                            st[:, hh * QC + js:(hh + 1) * QC],
                            krot[hh * 64:(hh + 1) * 64,
                                 kt * 128:(kt + 1) * 128],
                            qrot[hh * 64:(hh + 1) * 64, q0 + js:q0 + QC],
                            start=True, stop=True)
                    pt = wkb.tile([128, 2 * QC], BF16, tag="pt", bufs=4,
                                  name="pt")
                    if causal and j >= 0:
                        for hh in range(2):
                            nc.scalar.activation(
                                pt[:, hh * QC + j * 128:(hh + 1) * QC],
                                st[:, hh * QC + j * 128:(hh + 1) * QC],
                                mybir.ActivationFunctionType.Exp, scale=0.125)
                            nc.vector.tensor_mul(
                                pt[:, hh * QC + j * 128:
                                   hh * QC + (j + 1) * 128],
                                pt[:, hh * QC + j * 128:
                                   hh * QC + (j + 1) * 128], tri)
                    else:
                        nc.scalar.activation(
                            pt[:], st[:],
                            mybir.ActivationFunctionType.Exp, scale=0.125)
                    window.append((kt, pt))
                    if len(window) > LAG:
                        emit_pv(*window.pop(0), last=False)
                    if deferred and kt >= 3 and kt % 2 == 1:
                        deferred.pop(0)()
                    if fill:
                        for _ in range(per_kt):
                            if fill:
                                fill.pop(0)()
                while window:
                    kt_, p_ = window.pop(0)
                    emit_pv(kt_, p_, last=(kt_ == n_k - 1))
                while deferred:
                    deferred.pop(0)()
                # evict PV accumulators to SBUF, freeing the PSUM banks
                pvsb = []
                for hh in range(2):
                    pb = wkb.tile([65, QC], F32R, tag="pvsb", bufs=4,
                                  name="pvsb")
                    nc.vector.tensor_copy(pb[:], pvs[hh][:])
                    pvsb.append(pb)
                return pvsb

            sb0 = attention_chunk(0, [], fillers0, per_kt=2)
            sb1 = attention_chunk(1, norm_thunks(0, sb0), fillers, per_kt=2)
            d1 = norm_thunks(1, sb1)
            while fillers:
                fillers.pop(0)()
                if d1:
                    d1.pop(0)()
            sb2 = attention_chunk(2, d1, [])
            sb3 = attention_chunk(3, norm_thunks(2, sb2), [])
            for t in norm_thunks(3, sb3):
                t()

    _split_multi_waits(nc)
    return nc


_CONSTS = _host_constants()
_PROGRAMS = {}


def _get_program(causal: bool):
    if causal not in _PROGRAMS:
        _PROGRAMS[causal] = _build_program(causal)
    return _PROGRAMS[causal]


def _make_in_maps(x, w_in, w_out):
    x2 = np.asarray(x, dtype=np.float32).reshape(S, D)
    xT = np.ascontiguousarray(x2.T).astype(NPBF16)        # [D, S]
    # pack: xh[p, half*8192 + d*1024 + sl] = xT[d*128+p, half*1024+sl]
    xq = xT.reshape(ND, 128, 2, 1024).transpose(2, 1, 0, 3)  # [half,p,d,sl]
    xAh = np.ascontiguousarray(xq[0].reshape(128, ND * 1024))
    xBh = np.ascontiguousarray(xq[1].reshape(128, ND * 1024))
    w_in = np.asarray(w_in, dtype=np.float32)
    w_out = np.asarray(w_out, dtype=np.float32)

    in_maps = []
    for c in range(NCORES):
        r0 = c * EPC
        wq = w_in[r0:r0 + EPC, :]                          # [128, D]
        wk = w_in[D + r0:D + r0 + EPC, :]
        wv = w_in[2 * D + r0:2 * D + r0 + EPC, :]
        winT = np.ascontiguousarray(
            np.concatenate([wq, wk, wv], axis=0).T)        # [D, 384]
        winP = np.ascontiguousarray(
            winT.reshape(ND, 128, 3 * EPC).transpose(1, 0, 2)
            .reshape(128, ND * 3 * EPC)).astype(NPBF16)
        woP = np.ascontiguousarray(
            w_out[:, r0:r0 + EPC].T).astype(NPBF16)        # [128, D]
        in_maps.append({"xA": xAh, "xB": xBh, "winP": winP, "woP": woP,
                        "cos2": _CONSTS[0], "sin2": _CONSTS[1],
                        "consts": _CONSTS[2]})
    return in_maps


def kernel(x, w_in, w_out, is_causal):
    causal = bool(np.asarray(is_causal).item())
    nc = _get_program(causal)
    in_maps = _make_in_maps(x, w_in, w_out)
    res = run_bass_kernel_spmd(nc, in_maps, list(range(NCORES)))
    out = np.zeros((S, D), dtype=np.float64)
    for c in range(NCORES):
        out += res.results[c]["pout"].astype(np.float64)
    return out.astype(np.float32).reshape(B, S, D)
